# revision 39
# baseline (speedup 1.0000x reference)
"""Trainium2 Bass kernel for nn_BitwiseNetwork (STFT-style masking net).

Strategy (per core, data-parallel over batch, 4 batch items per core):
  - conv1 (1->1026ch, K=1024, S=256) as PE matmuls over a 128-sample "pieces"
    layout of x; stride-2 column APs give every frame without data duplication.
    in_gamma is folded into the conv1 weights on the host; tanh on ACT.
  - channels are permuted so real/imag cut pairs share partitions:
    tiles 0..7 each hold 4x(16 real + 16 imag) cuts; tile 8 = [r512, i512].
  - combine1 (2->8 per cut) as packed K=32 PE matmuls (block-diag weights),
    relu on ACT; combine2 (8->1) as K=128 scatter matmuls accumulated in PSUM.
  - linear 513x513 over cuts with output columns duplicated/interleaved so the
    mask tiles align 1:1 with the t tiles; mask = sigmoid(2g z + 2g b) 1 ACT op.
  - softmax(2ch) folded: out0 = sigmoid(convT_{w0-w1}(t*mask)), out1 = 1-out0.
    convT evaluated as 4 shifted PSUM-accumulated matmuls (frame overlap).
All matmuls run in float32r (~1e-4 rel err, ~4x faster than fp32 on PE).
"""

import numpy as np
import ml_dtypes

import concourse.bass as bass
import concourse.bacc as bacc
import concourse.mybir as mybir
import concourse.tile as tile
from concourse.bass_utils import run_bass_kernel_spmd

f32 = mybir.dt.float32
f32r = mybir.dt.float32r
bf16 = mybir.dt.bfloat16
AF = mybir.ActivationFunctionType
ALU = mybir.AluOpType

KSZ, STR, CUT, TC, CH = 1024, 256, 513, 1026, 8
N_CORES = 8


def _chunks(total, maxc):
    n = (total + maxc - 1) // maxc
    base, rem = divmod(total, n)
    out, pos = [], 0
    for i in range(n):
        c = base + (1 if i < rem else 0)
        out.append((pos, c))
        pos += c
    return out


def _perm():
    p = []
    for j in range(8):
        for blk in range(4):
            c0 = 64 * j + 16 * blk
            p += [c0 + t for t in range(16)]
            p += [513 + c0 + t for t in range(16)]
    p += [512, 513 + 512]
    return np.array(p)


def _kappa(j, p):
    return 64 * j + 16 * (p // 32) + (p % 32) % 16


def pack_weights(conv1_w, in_gamma, comb1_w, comb1_b, comb2_w, comb2_b,
                 lin_w, lin_b, fc_gamma, convT_w):
    perm = _perm()
    W1 = (np.asarray(conv1_w)[:, 0, :] * np.asarray(in_gamma)[:, None])[perm]
    wd = (np.asarray(convT_w)[:, 0, :] - np.asarray(convT_w)[:, 1, :])[perm]
    comb1_w = np.asarray(comb1_w); comb1_b = np.asarray(comb1_b)
    comb2_w = np.asarray(comb2_w); comb2_b = np.asarray(comb2_b)
    lin_w = np.asarray(lin_w); lin_b = np.asarray(lin_b)
    fc_gamma = np.asarray(fc_gamma)

    w1 = np.zeros((128, 8 * 8 * 128), np.float32)
    for i in range(8):
        for c in range(8):
            # lhsT[kk, m] = W1[128*i + m, 128*c + kk]
            w1[:, (i * 8 + c) * 128:(i * 8 + c + 1) * 128] = \
                W1[128 * i:128 * i + 128, 128 * c:128 * c + 128].T
    w1t8 = np.zeros((128, 1024), np.float32)
    for c in range(8):
        w1t8[:, 128 * c:128 * c + 2] = W1[1024:1026, 128 * c:128 * c + 128].T

    # combine1 as K=64 matmuls at partition bases {0, 64}; the 8 hidden units
    # are split into two halves of 4 so out fits 128 partitions (4o x 32x).
    # c1[:, 128*half : ...] = (64, 128) block replicated twice along partitions.
    c1 = np.zeros((128, 256), np.float32)
    for half in range(2):
        blk = np.zeros((64, 128), np.float32)
        for p in range(64):
            u, wv = p // 32, p % 32
            if wv < 16:
                xcut, which = 16 * u + wv, 0     # real row
            else:
                xcut, which = 16 * u + wv - 16, 1  # imag row
            for o in range(4):
                blk[p, 32 * o + xcut] = comb1_w[o + 4 * half, which]
        c1[0:64, 128 * half:128 * half + 128] = blk
        c1[64:128, 128 * half:128 * half + 128] = blk
    c1b = np.zeros((128, 2), np.float32)
    for half in range(2):
        for o in range(4):
            c1b[32 * o:32 * o + 32, half] = comb1_b[o + 4 * half]
    c18 = np.zeros((2, 128), np.float32)                   # padded to M=128
    c18[:, 0:8] = comb1_w.T
    c1b8 = comb1_b.astype(np.float32)[:, None]             # (8, 1)

    v2 = np.zeros((128, 8 * 128), np.float32)
    for jj in range(2):
        for bb in range(2):
            for half in range(2):
                g2 = (jj * 2 + bb) * 2 + half
                for o in range(4):
                    for xx in range(32):
                        v2[32 * o + xx,
                           128 * g2 + 64 * jj + 32 * bb + xx] = \
                            comb2_w[0, o + 4 * half]
    v28 = np.zeros((8, 128), np.float32)                   # padded to M=128
    v28[:, 0] = comb2_w[0]
    c2b = np.full((128, 1), comb2_b[0], np.float32)

    lina = np.zeros((128, 4 * 1152), np.float32)
    linb = np.zeros((1, 1152), np.float32)
    for a in range(4):
        for jt in range(8):
            kap = np.array([_kappa(jt, p) for p in range(128)])
            lina[:, a * 1152 + 128 * jt: a * 1152 + 128 * jt + 128] = \
                lin_w[kap, 128 * a:128 * a + 128].T
        lina[:, a * 1152 + 1024: a * 1152 + 1026] = \
            np.repeat(lin_w[512:513, 128 * a:128 * a + 128].T, 2, axis=1)
    for jt in range(8):
        kap = np.array([_kappa(jt, p) for p in range(128)])
        linb[0, 128 * jt:128 * jt + 128] = lin_w[kap, 512]
    linb[0, 1024:1026] = lin_w[512, 512]

    msc = np.zeros((128, 8), np.float32)
    mbi = np.zeros((128, 8), np.float32)
    for jt in range(8):
        kap = np.array([_kappa(jt, p) for p in range(128)])
        msc[:, jt] = 2.0 * fc_gamma[kap]
        mbi[:, jt] = 2.0 * fc_gamma[kap] * lin_b[kap]
    msc8 = np.full((2, 1), 2.0 * fc_gamma[512], np.float32)
    mbi8 = np.full((2, 1), 2.0 * fc_gamma[512] * lin_b[512], np.float32)

    wdt = np.zeros((128, 8 * 4 * 2 * 128), np.float32)
    for i in range(8):
        for jj in range(1, 5):
            for st in range(2):
                col = ((i * 4 + (jj - 1)) * 2 + st) * 128
                wdt[:, col:col + 128] = \
                    wd[128 * i:128 * i + 128,
                       (4 - jj) * 256 + 128 * st:(4 - jj) * 256 + 128 * st + 128]
    wdt8 = np.zeros((2, 4 * 2 * 128), np.float32)
    for jj in range(1, 5):
        for st in range(2):
            col = ((jj - 1) * 2 + st) * 128
            wdt8[:, col:col + 128] = \
                wd[1024:1026,
                   (4 - jj) * 256 + 128 * st:(4 - jj) * 256 + 128 * st + 128]

    d = dict(w1=w1, w1t8=w1t8, c1=c1, c1b=c1b, c18=c18, c1b8=c1b8,
             v2=v2, v28=v28, c2b=c2b, lina=lina, linb=linb,
             msc=msc, mbi=mbi, msc8=msc8, mbi8=mbi8, wdt=wdt, wdt8=wdt8,
             zpad=np.zeros((128, 10), np.float32))
    return d


_W_SHAPES = dict(w1=(128, 8192), w1t8=(128, 1024), c1=(128, 256),
                 c1b=(128, 2), c18=(2, 128), c1b8=(8, 1), v2=(128, 1024),
                 v28=(8, 128), c2b=(128, 1), lina=(128, 4608),
                 linb=(1, 1152), msc=(128, 8), mbi=(128, 8), msc8=(2, 1),
                 mbi8=(2, 1), wdt=(128, 8192), wdt8=(2, 1024), zpad=(128, 10))
_F32R_W = {"w1", "w1t8", "c1", "c18", "v2", "v28", "lina", "linb", "wdt", "wdt8", "zpad"}
_BF16_W = set()


def build_nc(T, BLOC, fch=343, t_bufs=9, h_bufs=4, loop_reps=1,
             stages=("conv1", "combine", "linear", "convt"),
             relu_eng="da", pbufs=(3, 3, 2), convt_even=False, nqc=256,
             combine8=True):
    P = T // 128
    NF = P // 2 + 3                 # frames used by convT: f = 1..NF
    NFP = NF + (NF & 1)             # padded even (fp32r needs even counts)
    NQ = T // 256
    fchunks = [(2 * c0, 2 * n) for (c0, n) in _chunks(NFP // 2, fch // 2)]
    qchunks = [(2 * c0, 2 * n) for (c0, n) in _chunks(NQ // 2, nqc // 2)]

    nc = bacc.Bacc("TRN2", target_bir_lowering=False, debug=False,
                   num_devices=N_CORES)
    x_ap = nc.dram_tensor("x", (BLOC, 128, P), f32, kind="ExternalInput").ap()
    y_ap = nc.dram_tensor("y", (BLOC, 2, 2, 128, NQ), f32,
                          kind="ExternalOutput").ap()
    w_aps = {k: nc.dram_tensor(k, s, bf16 if k in _BF16_W else f32,
                               kind="ExternalInput").ap()
             for k, s in _W_SHAPES.items()}

    fw = max(n for (_, n) in fchunks)

    with tile.TileContext(nc) as tc:
        with (tc.tile_pool(name="wpool", bufs=1) as wpool,
              tc.tile_pool(name="x2pool", bufs=2) as x2pool,
              tc.tile_pool(name="tpool", bufs=t_bufs) as tpool,
              tc.tile_pool(name="t8pool", bufs=1) as t8pool,
              tc.tile_pool(name="hpool", bufs=h_bufs) as hpool,
              tc.tile_pool(name="h8pool", bufs=1) as h8pool,
              tc.tile_pool(name="hgpool", bufs=16) as hgpool,
              tc.tile_pool(name="hg8pool", bufs=2) as hg8pool,
              tc.tile_pool(name="mpool", bufs=2) as mpool,
              tc.tile_pool(name="m8pool", bufs=2) as m8pool,
              tc.tile_pool(name="opool", bufs=2) as opool,
              tc.tile_pool(name="pmain", bufs=pbufs[0], space="PSUM") as pmain,
              tc.tile_pool(name="pp1", bufs=pbufs[1], space="PSUM") as pp1,
              tc.tile_pool(name="pp2", bufs=pbufs[2], space="PSUM") as pp2):

            wsb = {}
            for k, shp in _W_SHAPES.items():
                dt = (f32r if k in _F32R_W
                      else bf16 if k in _BF16_W else f32)
                wt = wpool.tile(list(shp), dt, name=f"w_{k}")
                src = w_aps[k][:]
                if k in _F32R_W:
                    src = src.bitcast(f32r)
                nc.sync.dma_start(wt[:], src)
                wsb[k] = wt

            def ps_tile(name):
                return pmain.tile([128, 512], f32, tag="psm", name=name)

            def emit_batch(b):
                # ---- load x into pieces layout
                X2 = x2pool.tile([128, P + 18], f32r, tag="x2", name=f"X2_{b}")
                nc.sync.dma_start(X2[:, 0:8],
                                  w_aps["zpad"][:, 0:8].bitcast(f32r))
                nc.sync.dma_start(X2[:, 8 + P:],
                                  w_aps["zpad"][:].bitcast(f32r))
                nc.sync.dma_start(
                    X2[:, 8:8 + P],
                    x_ap[b].bitcast(f32r))

                # ---- conv1 + tanh -> t tiles (f32r)
                tt = [tpool.tile([128, NFP], f32r, tag="t", name=f"t{b}_{j}")
                      for j in range(8)]
                t8 = t8pool.tile([2, NFP], f32r, tag="t8", name=f"t8_{b}")
                for i in range(9) if "conv1" in stages else []:
                    M = 128 if i < 8 else 2
                    for (c0, ncols) in fchunks:
                        f0 = c0 + 1
                        ps = ps_tile(f"psc_{b}_{i}_{c0}")
                        for c in range(8):
                            if i < 8:
                                lhsT = wsb["w1"][:, (i * 8 + c) * 128:
                                                 (i * 8 + c + 1) * 128]
                            else:
                                lhsT = wsb["w1t8"][:, 128 * c:128 * c + 128]
                            rhs = X2[:, 2 * f0 + c: 2 * f0 + c + 2 * ncols: 2]
                            nc.tensor.matmul(ps[:128, :ncols], lhsT, rhs,
                                             start=(c == 0), stop=(c == 7))
                        dst = tt[i] if i < 8 else t8
                        nc.scalar.activation(dst[:M, c0:c0 + ncols],
                                             ps[:M, :ncols], AF.Tanh)

                # ---- combine (2->8->1) -> h tiles (f32r)
                hh = [hpool.tile([128, NFP], f32r, tag="h", name=f"h{b}_{o}")
                      for o in range(4)]
                h8 = h8pool.tile([1, NFP], f32r, tag="h8", name=f"h8_{b}")
                for (c0, ncols) in fchunks if "combine" in stages else []:
                    # combine1 (relu split DVE/ACT) and combine2 matmul
                    # batches interleaved at ot granularity: PE stream
                    # stays dense ([mm1 x8][mm2 x8 prev ot][mm1 x8]...),
                    # relus run one ot batch behind on DVE/ACT.
                    def emit_mm2(ot, hgs_ot):
                        ps2 = pp2.tile([128, 512], f32, tag="ps2",
                                       name=f"ps2_{b}_{ot}_{c0}")
                        for g2 in range(8):
                            nc.tensor.matmul(
                                ps2[:128, :ncols],
                                wsb["v2"][:, 128 * g2:128 * g2 + 128],
                                hgs_ot[g2][:, :ncols],
                                start=(g2 == 0), stop=(g2 == 7))
                        nc.scalar.activation(hh[ot][:, c0:c0 + ncols],
                                             ps2[:128, :ncols], AF.Relu,
                                             bias=wsb["c2b"][:, 0:1])

                    pend = None
                    for ot in range(4):
                        hgs_ot = []
                        for g2 in range(8):
                            u = ot * 8 + g2
                            jj, bb, half = g2 // 4, (g2 // 2) % 2, g2 % 2
                            j, beta = 2 * ot + jj, 64 * bb
                            ps1 = pp1.tile([128, 512], f32, tag="ps1",
                                           name=f"ps1_{b}_{c0}_{u}")
                            nc.tensor.matmul(
                                ps1[:128, :ncols],
                                wsb["c1"][beta:beta + 64,
                                          128 * half:128 * half + 128],
                                tt[j][beta:beta + 64, c0:c0 + ncols],
                                start=True, stop=True)
                            hg = hgpool.tile([128, fw], f32r, tag="hg",
                                             name=f"hg_{b}_{c0}_{u}")
                            eng = relu_eng[u % len(relu_eng)]
                            if eng == "d":
                                nc.vector.tensor_scalar(
                                    hg[:, :ncols], ps1[:128, :ncols],
                                    wsb["c1b"][:, half:half + 1], 0.0,
                                    ALU.add, ALU.max)
                            elif eng == "p":
                                nc.gpsimd.tensor_scalar(
                                    hg[:, :ncols], ps1[:128, :ncols],
                                    wsb["c1b"][:, half:half + 1], 0.0,
                                    ALU.add, ALU.max)
                            else:
                                nc.scalar.activation(
                                    hg[:, :ncols], ps1[:128, :ncols],
                                    AF.Relu,
                                    bias=wsb["c1b"][:, half:half + 1])
                            hgs_ot.append(hg)
                        if pend is not None:
                            emit_mm2(*pend)
                        pend = (ot, hgs_ot)
                    emit_mm2(*pend)
                    # pair-512 combine
                    if not combine8:
                        continue
                    ps1b = ps_tile(f"ps1b_{b}_{c0}")
                    nc.tensor.matmul(ps1b[:128, :ncols], wsb["c18"][:],
                                     t8[0:2, c0:c0 + ncols],
                                     start=True, stop=True)
                    hg8 = hg8pool.tile([8, fw], f32r, tag="hg8",
                                       name=f"hg8_{b}_{c0}")
                    nc.scalar.activation(hg8[:, :ncols], ps1b[:8, :ncols],
                                         AF.Relu, bias=wsb["c1b8"][:, 0:1])
                    psh8 = ps_tile(f"psh8_{b}_{c0}")
                    nc.tensor.matmul(psh8[:128, :ncols], wsb["v28"][:],
                                     hg8[:, :ncols], start=True, stop=True)
                    nc.scalar.activation(h8[0:1, c0:c0 + ncols],
                                         psh8[:1, :ncols], AF.Relu,
                                         bias=wsb["c2b"][0:1, 0:1])

                # ---- dup-linear + mask + apply to t (in place)
                for (c0, ncols) in fchunks if "linear" in stages else []:
                    for jt in range(9):
                        M = 128 if jt < 8 else 2
                        ps3 = ps_tile(f"ps3_{b}_{jt}_{c0}")
                        for a in range(5):
                            if a < 4:
                                lhsT = wsb["lina"][:, a * 1152 + 128 * jt:
                                                   a * 1152 + 128 * jt + 128]
                                rhs = hh[a][:, c0:c0 + ncols]
                            else:
                                lhsT = wsb["linb"][0:1,
                                                   128 * jt:128 * jt + 128]
                                rhs = h8[0:1, c0:c0 + ncols]
                            nc.tensor.matmul(ps3[:128, :ncols], lhsT, rhs,
                                             start=(a == 0), stop=(a == 4))
                        if jt < 8:
                            mk = mpool.tile([128, fw], f32r, tag="m",
                                            name=f"mk_{b}_{jt}_{c0}")
                            nc.scalar.activation(
                                mk[:, :ncols], ps3[:128, :ncols], AF.Sigmoid,
                                bias=wsb["mbi"][:, jt:jt + 1],
                                scale=wsb["msc"][:, jt:jt + 1])
                            nc.vector.tensor_mul(tt[jt][:, c0:c0 + ncols],
                                                 tt[jt][:, c0:c0 + ncols],
                                                 mk[:, :ncols])
                        else:
                            mk8 = m8pool.tile([2, fw], f32r, tag="m8",
                                              name=f"mk8_{b}_{c0}")
                            nc.scalar.activation(
                                mk8[:, :ncols], ps3[:2, :ncols], AF.Sigmoid,
                                bias=wsb["mbi8"][:, 0:1],
                                scale=wsb["msc8"][:, 0:1])
                            nc.vector.tensor_mul(t8[:, c0:c0 + ncols],
                                                 t8[:, c0:c0 + ncols],
                                                 mk8[:, :ncols])

                # ---- convT (folded to y0-y1) + sigmoid -> outputs
                for st in range(2) if "convt" in stages else []:
                    for (q0, nq) in qchunks:
                        ps4 = ps_tile(f"ps4_{b}_{st}_{q0}")
                        idx = 0
                        for jj in range(1, 5):
                            for i in range(9):
                                qo = (q0 + jj - 1) & ~1 if convt_even \
                                    else q0 + jj - 1
                                if i < 8:
                                    col = ((i * 4 + (jj - 1)) * 2 + st) * 128
                                    lhsT = wsb["wdt"][:, col:col + 128]
                                    rhs = tt[i][:, qo:qo + nq]
                                else:
                                    col = ((jj - 1) * 2 + st) * 128
                                    lhsT = wsb["wdt8"][:, col:col + 128]
                                    rhs = t8[:, qo:qo + nq]
                                nc.tensor.matmul(ps4[:128, :nq], lhsT, rhs,
                                                 start=(idx == 0),
                                                 stop=(idx == 35))
                                idx += 1
                        o0 = opool.tile([128, 512], f32, tag="o",
                                        name=f"o0_{b}_{st}_{q0}")
                        nc.scalar.activation(o0[:, :nq], ps4[:128, :nq],
                                             AF.Sigmoid)
                        o1 = opool.tile([128, 512], f32, tag="o",
                                        name=f"o1_{b}_{st}_{q0}")
                        nc.vector.tensor_scalar(
                            o1[:, :nq], o0[:, :nq], -1.0, 1.0,
                            mybir.AluOpType.mult, mybir.AluOpType.add)
                        nc.sync.dma_start(
                            y_ap[b, 0, st, :, q0:q0 + nq], o0[:, :nq])
                        nc.sync.dma_start(
                            y_ap[b, 1, st, :, q0:q0 + nq], o1[:, :nq])

            if loop_reps == 1:
                for b in range(BLOC):
                    emit_batch(b)
            else:
                with tc.For_i(0, loop_reps, 1):
                    for b in range(BLOC):
                        emit_batch(b)
    nc.compile()
    return nc


_NC_CACHE = {}


def _get_nc(T, BLOC):
    key = (T, BLOC)
    if key not in _NC_CACHE:
        _NC_CACHE[key] = build_nc(T, BLOC)
    return _NC_CACHE[key]


def kernel(x, conv1_w, in_gamma, comb1_w, comb1_b, comb2_w, comb2_b,
           lin_w, lin_b, fc_gamma, convT_w):
    x = np.asarray(x)
    B, _, T = x.shape
    BLOC = B // N_CORES
    P = T // 128
    NQ = T // 256
    nc = _get_nc(T, BLOC)
    w = pack_weights(conv1_w, in_gamma, comb1_w, comb1_b, comb2_w, comb2_b,
                     lin_w, lin_b, fc_gamma, convT_w)
    in_maps = []
    for core in range(N_CORES):
        shard = x[core * BLOC:(core + 1) * BLOC, 0, :]
        m = {"x": np.ascontiguousarray(
            shard.reshape(BLOC, P, 128).transpose(0, 2, 1))}
        m.update(w)
        in_maps.append(m)
    res = run_bass_kernel_spmd(nc, in_maps, core_ids=list(range(N_CORES)))
    outs = [r["y"].reshape(BLOC, 2, 2, 128, NQ).transpose(0, 1, 4, 2, 3)
            .reshape(BLOC, 2, T) for r in res.results]
    return np.concatenate(outs, axis=0)

